# revision 6
# baseline (speedup 1.0000x reference)
"""Trainium2 Bass kernel for nn_DechunkingLayer (ragged_sequence).

Reference semantics (per batch row):
    idx = clip(exclusive_cumsum(b), 0, NC - 1)          # [T]
    up[t]  = z[idx[t]]                                  # gather rows
    out[t] = p[t] * up[t] + (1 - p[t]) * up[t-1]        # EMA blend
    out[0] = up[0]

Sharding: pure data parallel over batch B=8 across the 8 NeuronCores
(one batch row per core). All work per row is independent.

Per-core plan (HBM traffic = 16 MB gather + 16 MB store = 32 MB):
  - The kernel is DMA-engine-time bound: ~44-63us/engine for the
    4096-row indirect gather (168-228ns per 4KB random read, depending
    on cross-core HBM contention) + ~41us/engine for the stores (160ns
    per 4KB write, at the HBM cap). Compute hides under that; only the
    startup latency (idx chain before the gather stream saturates) and
    the tail are recoverable wall-clock.
  - Tile-0 fast path: b[0:128] loaded as a [128,1] column, one
    triangular matmul, and the first gather issues ~4us early.
  - Full idx in W layout [128, 32] (partition = t % 128, column =
    t // 128) with a short dependency chain: b is DMA'd directly into
    W layout (4B-descriptor transposing load, no PE transpose), the
    partition-scan (tri @ b_w) and the column-offset broadcast
    (allones @ (b_nat @ tri32)) accumulate into one PSUM tile.
  - rolled (up[t-1]) inside a tile is the gathered tile shifted down
    one partition via a shifted-identity PE matmul (bitwise exact; the
    blend cancels to near-zero outputs so every value-path op must be
    exact f32). The last 2 tiles instead HBM-gather rolled so the
    final stores don't queue behind the PE matmul backlog.
  - per-tile rows t=128k blend against the previous tile's last row;
    those 32 rows are redone exactly in a small epilogue whose store
    rides the same HWDGE queue as the main stores (FIFO overwrite).
  - out[0] = up[0] exactly via forcing p[0] = 1 (q[0] = 0).
"""

import numpy as np

import concourse.bacc as bacc
import concourse.bass as bass
import concourse.mybir as mybir
import concourse.tile as tile
from concourse.bass import IndirectOffsetOnAxis
from concourse.bass_utils import run_bass_kernel_spmd
from concourse.masks import make_identity, make_upper_triangular

# Problem shape (hardcoded per harness contract).
B = 8          # batch rows == number of cores
T = 4096       # timesteps per row
NCH = 2048     # number of chunks (z rows)
D = 1024       # d_model
P = 128        # SBUF partitions
NT = T // P    # 32 tiles per core
NCOL = T // P  # 32 columns in the W layout
DH = D // 2    # matmul free-dim max for fp32 is 512
NTAIL = 2      # trailing tiles whose `rolled` comes from an HBM gather

F32 = mybir.dt.float32
I32 = mybir.dt.int32

WARMUP_MM = 10  # PE warm-up matmuls to release the HAM clock throttle


def build_bass() -> bass.Bass:
    # Bacc (not raw Bass): its finalize() runs generate_event_semaphores,
    # which splits multi-sem waits to satisfy TRN2's one-wait-per-instruction
    # ISA constraint.
    nc = bacc.Bacc()

    z = nc.dram_tensor("z", [NCH, D], F32, kind="ExternalInput")
    p = nc.dram_tensor("p", [T], F32, kind="ExternalInput")
    b = nc.dram_tensor("b", [T], I32, kind="ExternalInput")
    out = nc.dram_tensor("out", [T, D], F32, kind="ExternalOutput")

    with tile.TileContext(nc) as tc:
        with (
            tc.tile_pool(name="setup", bufs=1) as sp,
            tc.tile_pool(name="psmall", bufs=2, space="PSUM") as pps,
            tc.tile_pool(name="proll", bufs=3, space="PSUM") as ppr,
            tc.tile_pool(name="main", bufs=5) as mp,
        ):
            # ---- gpsimd constant builds (front-loaded) ---------------------
            # affine_select only exists on gpsimd; the gpsimd queue is
            # in-order, so every gpsimd-built constant must be enqueued
            # BEFORE the first indirect gather (whose semaphore wait would
            # otherwise stall the builds). PE Matmult has a single sync-wait
            # slot, so launder every matmul operand through DVE so all
            # matmul waits collapse onto one DVE semaphore.
            tri_g = sp.tile([P, P], F32)     # tri[k, i] = 1 iff i > k
            make_upper_triangular(nc, tri_g[:], val=1.0, diag=False)
            tri = sp.tile([P, P], F32)
            nc.vector.tensor_copy(out=tri[:], in_=tri_g[:])

            tri32_g = sp.tile([NCOL, NCOL], F32)  # [k, j] = 1 iff j > k
            make_upper_triangular(nc, tri32_g[:], val=1.0, diag=False)
            tri32 = sp.tile([NCOL, NCOL], F32)
            nc.vector.tensor_copy(out=tri32[:], in_=tri32_g[:])

            # shifted identity: S[k, i] = 1 iff i == k + 1  ->  (S^T @ x)[i] = x[i-1]
            ish_g = sp.tile([P, P], F32)
            nc.gpsimd.memset(ish_g[:], 0.0)
            nc.gpsimd.affine_select(
                out=ish_g[:], in_=ish_g[:],
                compare_op=mybir.AluOpType.not_equal, fill=1.0,
                base=1, pattern=[[-1, P]], channel_multiplier=1,
            )
            ishift = sp.tile([P, P], F32)
            nc.vector.tensor_copy(out=ishift[:], in_=ish_g[:])

            identf_g = sp.tile([NCOL, NCOL], F32)
            make_identity(nc, identf_g[:])
            identf = sp.tile([NCOL, NCOL], F32)
            nc.vector.tensor_copy(out=identf[:], in_=identf_g[:])

            # ---- tile-0 fast path ------------------------------------------
            # idx for t<128 is the exclusive cumsum of b[0:128]; load it as a
            # [128, 1] column and do a single triangular matmul -- the first
            # gather issues well before the full W-layout chain resolves.
            bcol0_i = sp.tile([P, 1], I32)
            nc.sync.dma_start(out=bcol0_i[:], in_=b[:].rearrange("(c o) -> c o", o=1)[0:P])
            bcol0 = sp.tile([P, 1], F32)
            nc.vector.tensor_copy(out=bcol0[:], in_=bcol0_i[:])

            s0_ps = pps.tile([P, 1], F32, space="PSUM", tag="small_ps")
            nc.tensor.matmul(out=s0_ps[:], lhsT=tri[:], rhs=bcol0[:],
                             start=True, stop=True)
            # t<128 => idx < 128 <= NCH-1, no clamp needed on the fast path
            idx0_i = sp.tile([P, 1], I32)
            nc.vector.tensor_copy(out=idx0_i[:], in_=s0_ps[:])

            up0 = mp.tile([P, D], F32, tag="up")
            nc.gpsimd.indirect_dma_start(
                out=up0[:], out_offset=None, in_=z[:],
                in_offset=IndirectOffsetOnAxis(ap=idx0_i[:, 0:1], axis=0),
            )

            # ---- load b straight into W layout + natural, and p ------------
            # bw_i[c, j] = b[128j + c]: a 4B-descriptor transposing load;
            # ~4096 tiny descriptors cost ~0.2us/engine, and skipping the PE
            # transpose shortens the idx dependency chain by two hops.
            bw_i = sp.tile([P, NCOL], I32)
            nc.sync.dma_start(out=bw_i[:], in_=b[:].rearrange("(j c) -> c j", c=P))
            b2d = b[:].rearrange("(j c) -> j c", c=P)          # [32, 128] DRAM view
            b_nat_i = sp.tile([NCOL, P], I32)
            nc.sync.dma_start(out=b_nat_i[:], in_=b2d)
            # bprev2[r, u] = b[128*(NT-NTAIL+u) + r - 1] for the tail tiles'
            # rolled-gather indices (idx[t-1] = s[t] - b[t-1])
            bprev2_i = sp.tile([P, NTAIL], I32)
            nc.sync.dma_start(
                out=bprev2_i[:],
                in_=b[(NT - NTAIL) * P - 1 : NT * P - 1].rearrange(
                    "(u c) -> c u", c=P
                ),
            )
            p2d = p[:].rearrange("(j c) -> j c", c=P)
            p_nat = sp.tile([NCOL, P], F32)
            nc.sync.dma_start(out=p_nat[:], in_=p2d)

            b_w = sp.tile([P, NCOL], F32)
            nc.vector.tensor_copy(out=b_w[:], in_=bw_i[:])
            b_nat = sp.tile([NCOL, P], F32)
            nc.vector.tensor_copy(out=b_nat[:], in_=b_nat_i[:])
            bprev2 = sp.tile([P, NTAIL], F32)
            nc.vector.tensor_copy(out=bprev2[:], in_=bprev2_i[:])

            allones = sp.tile([P, P], F32)
            nc.vector.memset(allones[:], 1.0)
            ones_row_f = sp.tile([1, P], F32)
            nc.vector.memset(ones_row_f[:], 1.0)

            # ---- s = partition-scan + column-offset broadcast (one PSUM) ---
            # M[i, j] = sum_{j' < j} b[128j' + i]  (contraction over the 32
            # natural-layout partitions), then colofs[j] = sum_i M[i, j],
            # broadcast to all partitions by an all-ones lhsT.
            m_ps = pps.tile([P, NCOL], F32, space="PSUM", tag="small_ps")
            nc.tensor.matmul(out=m_ps[:], lhsT=b_nat[:], rhs=tri32[:],
                             start=True, stop=True)
            m_sb = sp.tile([P, NCOL], F32)
            nc.vector.tensor_copy(out=m_sb[:], in_=m_ps[:])

            s_ps = pps.tile([P, NCOL], F32, space="PSUM", tag="small_ps")
            nc.tensor.matmul(out=s_ps[:], lhsT=tri[:], rhs=b_w[:],
                             start=True, stop=False)
            nc.tensor.matmul(out=s_ps[:], lhsT=allones[:], rhs=m_sb[:],
                             start=False, stop=True)

            # ---- gather indices: idx = min(s, NCH-1) -----------------------
            idx_i = sp.tile([P, NCOL], I32)
            nc.vector.tensor_scalar_min(out=idx_i[:], in0=s_ps[:],
                                        scalar1=float(NCH - 1))
            idx_f = sp.tile([P, NCOL], F32)
            nc.vector.tensor_scalar_min(out=idx_f[:], in0=s_ps[:],
                                        scalar1=float(NCH - 1))

            # tail tiles: idx_prev = min(s - b_prev, NCH-1) (s[t]-b[t-1]=s[t-1])
            sprev = sp.tile([P, NTAIL], F32)
            nc.vector.tensor_sub(out=sprev[:], in0=s_ps[:, NT - NTAIL : NT],
                                 in1=bprev2[:])
            idxp_i = sp.tile([P, NTAIL], I32)
            nc.vector.tensor_scalar_min(out=idxp_i[:], in0=sprev[:],
                                        scalar1=float(NCH - 1))

            # ---- p / q in W layout (off the gather critical path) ----------
            p_nat_l = sp.tile([NCOL, P], F32)
            nc.vector.tensor_copy(out=p_nat_l[:], in_=p_nat[:])
            pwt_ps = pps.tile([P, NCOL], F32, space="PSUM", tag="small_ps")
            nc.tensor.transpose(out=pwt_ps[:], in_=p_nat_l[:], identity=identf[:])
            p_w = sp.tile([P, NCOL], F32)
            nc.vector.tensor_copy(out=p_w[:], in_=pwt_ps[:])
            # out[0] = up[0] exactly: force p[0] = 1 so the blend is 1*up + 0*rolled
            nc.vector.memset(p_w[0:1, 0:1], 1.0)
            q_w = sp.tile([P, NCOL], F32)  # q = 1 - p
            nc.scalar.activation(
                out=q_w[:], in_=p_w[:],
                func=mybir.ActivationFunctionType.Copy, bias=1.0, scale=-1.0,
            )

            # ---- epilogue vectors for rows t = 128j ------------------------
            # bprev_row[j] = idx[128j - 1] (0 for j=0, harmless: q[0]=0).
            # Row 127 of idx_f is not a legal compute-engine base, so extract
            # it with a tiny SBUF->SBUF DMA, then rotate rows into columns
            # with [1,32]-lhsT matmuls against a single 1.0.
            bprev_row = sp.tile([1, NCOL], F32)
            nc.vector.memset(bprev_row[:], 0.0)
            nc.sync.dma_start(
                out=bprev_row[0:1, 1:NCOL], in_=idx_f[P - 1 : P, 0 : NCOL - 1]
            )

            cols_ps = pps.tile([NCOL, 4], F32, space="PSUM", tag="small_ps")
            for ci, row in enumerate([bprev_row, idx_f, p_w, q_w]):
                nc.tensor.matmul(
                    out=cols_ps[:, ci : ci + 1],
                    lhsT=row[0:1, 0:NCOL],
                    rhs=ones_row_f[0:1, 0:1],
                    start=True, stop=True,
                )
            bidx_i = sp.tile([NCOL, 1], I32)
            nc.vector.tensor_copy(out=bidx_i[:], in_=cols_ps[:, 0:1])
            fidx_i = sp.tile([NCOL, 1], I32)
            nc.vector.tensor_copy(out=fidx_i[:], in_=cols_ps[:, 1:2])
            pb_col = sp.tile([NCOL, 1], F32)
            nc.vector.tensor_copy(out=pb_col[:], in_=cols_ps[:, 2:3])
            qb_col = sp.tile([NCOL, 1], F32)
            nc.vector.tensor_copy(out=qb_col[:], in_=cols_ps[:, 3:4])

            # PE warm-up: the HAM clock gate keeps the PE at ~half clock
            # until it has been busy for ~4us. Burn that in at the tail of
            # setup, while the PE would otherwise idle waiting for the first
            # gather, so the main-loop matmuls run at full clock.
            warm_src = sp.tile([P, DH], F32)
            nc.vector.memset(warm_src[:], 1.0)
            for w in range(WARMUP_MM):
                wps = ppr.tile([P, DH], F32, space="PSUM", tag="roll")
                nc.tensor.matmul(out=wps[:], lhsT=ishift[:], rhs=warm_src[:],
                                 start=True, stop=True, skip_group_check=True)
                if w == WARMUP_MM - 1:
                    warm_sink = sp.tile([1, 1], F32)
                    nc.vector.tensor_copy(out=warm_sink[:], in_=wps[0:1, 0:1])

            # ---- main loop: gather, roll, blend, store ---------------------
            rolled_tail = {}
            for k in range(NT):
                if k == 0:
                    up = up0
                else:
                    up = mp.tile([P, D], F32, tag="up")
                    nc.gpsimd.indirect_dma_start(
                        out=up[:], out_offset=None, in_=z[:],
                        in_offset=IndirectOffsetOnAxis(ap=idx_i[:, k : k + 1], axis=0),
                    )

                # t1 = p * up on ACT (exact f32)
                t1 = mp.tile([P, D], F32, tag="t1")
                nc.scalar.mul(out=t1[:], in_=up[:], mul=p_w[:, k : k + 1])

                o = mp.tile([P, D], F32, tag="o")
                if k >= NT - NTAIL:
                    # tail tiles: rolled was HBM-gathered mid-stream so the
                    # final stores don't queue behind the PE matmul backlog
                    nc.vector.scalar_tensor_tensor(
                        out=o[:], in0=rolled_tail[k][:], scalar=q_w[:, k : k + 1],
                        in1=t1[:],
                        op0=mybir.AluOpType.mult, op1=mybir.AluOpType.add,
                    )
                else:
                    # rolled[i] = up[i-1] via PE shifted-identity (f32, exact;
                    # row 0 -> 0, fixed by the epilogue)
                    rps = ppr.tile([P, D], F32, space="PSUM", tag="roll")
                    for h in range(2):
                        sl = slice(h * DH, (h + 1) * DH)
                        nc.tensor.matmul(out=rps[:, sl], lhsT=ishift[:],
                                         rhs=up[:, sl],
                                         start=True, stop=True,
                                         skip_group_check=True)
                    nc.vector.scalar_tensor_tensor(
                        out=o[:], in0=rps[:], scalar=q_w[:, k : k + 1],
                        in1=t1[:],
                        op0=mybir.AluOpType.mult, op1=mybir.AluOpType.add,
                    )

                nc.sync.dma_start(out=out[k * P : (k + 1) * P, :], in_=o[:])

                if k == 8:
                    # epilogue gathers + blend, issued mid-loop so they fill
                    # gather-stream slack instead of delaying tile 0 (gpsimd
                    # FIFO) or extending the tail; only the store is last.
                    upf = sp.tile([NCOL, D], F32)
                    nc.gpsimd.indirect_dma_start(
                        out=upf[:], out_offset=None, in_=z[:],
                        in_offset=IndirectOffsetOnAxis(ap=fidx_i[:, 0:1], axis=0),
                    )
                    rollf = sp.tile([NCOL, D], F32)
                    nc.gpsimd.indirect_dma_start(
                        out=rollf[:], out_offset=None, in_=z[:],
                        in_offset=IndirectOffsetOnAxis(ap=bidx_i[:, 0:1], axis=0),
                    )
                    t1b = sp.tile([NCOL, D], F32)
                    nc.scalar.mul(out=t1b[:], in_=upf[:], mul=pb_col[:])
                    ob = sp.tile([NCOL, D], F32)
                    nc.vector.scalar_tensor_tensor(
                        out=ob[:], in0=rollf[:], scalar=qb_col[:], in1=t1b[:],
                        op0=mybir.AluOpType.mult, op1=mybir.AluOpType.add,
                    )
                if k == 10:
                    # tail rolled-gathers, issued mid-stream: the data sits in
                    # SBUF until tiles NT-NTAIL.. blend against it.
                    for u in range(NTAIL):
                        rt = sp.tile([P, D], F32)
                        nc.gpsimd.indirect_dma_start(
                            out=rt[:], out_offset=None, in_=z[:],
                            in_offset=IndirectOffsetOnAxis(
                                ap=idxp_i[:, u : u + 1], axis=0
                            ),
                        )
                        rolled_tail[NT - NTAIL + u] = rt

            # ---- epilogue store: redo rows t = 128j exactly ----------------
            # Same HWDGE queue as the main stores, so FIFO order makes this
            # overwrite win.
            out_rows0 = out[:].rearrange("(j r) d -> j r d", r=P)[:, 0:1, :]
            nc.sync.dma_start(out=out_rows0, in_=ob[:, None, :])

    # Run the Bacc lowering passes (register allocation, event-semaphore
    # splitting, ...) -- run_bass_via_pjrt serializes nc.m as-is.
    nc.finalize()
    return nc


_NC_CACHE = None


def _get_nc() -> bass.Bass:
    global _NC_CACHE
    if _NC_CACHE is None:
        _NC_CACHE = build_bass()
    return _NC_CACHE


def make_in_maps(z: np.ndarray, p: np.ndarray, b: np.ndarray) -> list[dict]:
    return [
        {
            "z": np.ascontiguousarray(z[i], dtype=np.float32),
            "p": np.ascontiguousarray(p[i], dtype=np.float32),
            "b": np.ascontiguousarray(b[i], dtype=np.int32),
        }
        for i in range(B)
    ]


def kernel(z, p, b, original_len=None, **_unused) -> np.ndarray:
    z = np.asarray(z, dtype=np.float32)
    p = np.asarray(p, dtype=np.float32)
    b = np.asarray(b, dtype=np.int32)
    assert z.shape == (B, NCH, D) and p.shape == (B, T) and b.shape == (B, T)

    nc = _get_nc()
    res = run_bass_kernel_spmd(nc, make_in_maps(z, p, b), list(range(B)))
    return np.stack([r["out"] for r in res.results], axis=0)


# revision 7
# speedup vs baseline: 1.1626x; 1.1626x over previous
"""Trainium2 Bass kernel for nn_DechunkingLayer (ragged_sequence).

Reference semantics (per batch row):
    idx = clip(exclusive_cumsum(b), 0, NC - 1)          # [T]
    up[t]  = z[idx[t]]                                  # gather rows
    out[t] = p[t] * up[t] + (1 - p[t]) * up[t-1]        # EMA blend
    out[0] = up[0]

Sharding: pure data parallel over batch B=8 across the 8 NeuronCores
(one batch row per core). All work per row is independent.

Per-core plan (HBM traffic = 16 MB gather + 16 MB store = 32 MB):
  - The kernel is DMA-engine-time bound: ~44-63us/engine for the
    4096-row indirect gather (168-228ns per 4KB random read, depending
    on cross-core HBM contention) + ~41us/engine for the stores (160ns
    per 4KB write, at the HBM cap). Compute hides under that; only the
    startup latency (idx chain before the gather stream saturates) and
    the tail are recoverable wall-clock.
  - Tile-0 fast path: b[0:128] loaded as a [128,1] column, one
    triangular matmul, and the first gather issues ~4us early.
  - Full idx in W layout [128, 32] (partition = t % 128, column =
    t // 128) with a short dependency chain: b is DMA'd directly into
    W layout (4B-descriptor transposing load, no PE transpose), the
    partition-scan (tri @ b_w) and the column-offset broadcast
    (allones @ (b_nat @ tri32)) accumulate into one PSUM tile.
  - rolled (up[t-1]) inside a tile is the gathered tile shifted down
    one partition via a shifted-identity PE matmul (bitwise exact; the
    blend cancels to near-zero outputs so every value-path op must be
    exact f32). The last 2 tiles instead HBM-gather rolled so the
    final stores don't queue behind the PE matmul backlog.
  - per-tile rows t=128k blend against the previous tile's last row;
    those 32 rows are redone exactly in a small epilogue whose store
    rides the same HWDGE queue as the main stores (FIFO overwrite).
  - out[0] = up[0] exactly via forcing p[0] = 1 (q[0] = 0).
"""

import numpy as np

import concourse.bacc as bacc
import concourse.bass as bass
import concourse.mybir as mybir
import concourse.tile as tile
from concourse.bass import IndirectOffsetOnAxis
from concourse.bass_utils import run_bass_kernel_spmd
from concourse.masks import make_identity, make_upper_triangular

# Problem shape (hardcoded per harness contract).
B = 8          # batch rows == number of cores
T = 4096       # timesteps per row
NCH = 2048     # number of chunks (z rows)
D = 1024       # d_model
P = 128        # SBUF partitions
NT = T // P    # 32 tiles per core
NCOL = T // P  # 32 columns in the W layout
DH = D // 2    # matmul free-dim max for fp32 is 512
NTAIL = 2      # trailing tiles whose `rolled` comes from an HBM gather

F32 = mybir.dt.float32
I32 = mybir.dt.int32

WARMUP_MM = 10  # PE warm-up matmuls to release the HAM clock throttle


def build_bass() -> bass.Bass:
    # Bacc (not raw Bass): its finalize() runs generate_event_semaphores,
    # which splits multi-sem waits to satisfy TRN2's one-wait-per-instruction
    # ISA constraint.
    nc = bacc.Bacc()

    z = nc.dram_tensor("z", [NCH, D], F32, kind="ExternalInput")
    p = nc.dram_tensor("p", [T], F32, kind="ExternalInput")
    b = nc.dram_tensor("b", [T], I32, kind="ExternalInput")
    out = nc.dram_tensor("out", [T, D], F32, kind="ExternalOutput")

    with tile.TileContext(nc) as tc:
        with (
            tc.tile_pool(name="setup", bufs=1) as sp,
            tc.tile_pool(name="psmall", bufs=2, space="PSUM") as pps,
            tc.tile_pool(name="proll", bufs=3, space="PSUM") as ppr,
            tc.tile_pool(name="main", bufs=5) as mp,
        ):
            # ---- gpsimd constant builds (front-loaded) ---------------------
            # affine_select only exists on gpsimd; the gpsimd queue is
            # in-order, so every gpsimd-built constant must be enqueued
            # BEFORE the first indirect gather (whose semaphore wait would
            # otherwise stall the builds). PE Matmult has a single sync-wait
            # slot, so launder every matmul operand through DVE so all
            # matmul waits collapse onto one DVE semaphore.
            tri_g = sp.tile([P, P], F32)     # tri[k, i] = 1 iff i > k
            make_upper_triangular(nc, tri_g[:], val=1.0, diag=False)
            tri = sp.tile([P, P], F32)
            nc.vector.tensor_copy(out=tri[:], in_=tri_g[:])

            tri32_g = sp.tile([NCOL, NCOL], F32)  # [k, j] = 1 iff j > k
            make_upper_triangular(nc, tri32_g[:], val=1.0, diag=False)
            tri32 = sp.tile([NCOL, NCOL], F32)
            nc.vector.tensor_copy(out=tri32[:], in_=tri32_g[:])

            # shifted identity: S[k, i] = 1 iff i == k + 1  ->  (S^T @ x)[i] = x[i-1]
            ish_g = sp.tile([P, P], F32)
            nc.gpsimd.memset(ish_g[:], 0.0)
            nc.gpsimd.affine_select(
                out=ish_g[:], in_=ish_g[:],
                compare_op=mybir.AluOpType.not_equal, fill=1.0,
                base=1, pattern=[[-1, P]], channel_multiplier=1,
            )
            ishift = sp.tile([P, P], F32)
            nc.vector.tensor_copy(out=ishift[:], in_=ish_g[:])

            identf_g = sp.tile([NCOL, NCOL], F32)
            make_identity(nc, identf_g[:])
            identf = sp.tile([NCOL, NCOL], F32)
            nc.vector.tensor_copy(out=identf[:], in_=identf_g[:])

            # ---- tile-0 fast path ------------------------------------------
            # idx for t<128 is the exclusive cumsum of b[0:128]; load it as a
            # [128, 1] column and do a single triangular matmul -- the first
            # gather issues well before the full W-layout chain resolves.
            bcol0_i = sp.tile([P, 1], I32)
            nc.sync.dma_start(out=bcol0_i[:], in_=b[:].rearrange("(c o) -> c o", o=1)[0:P])
            bcol0 = sp.tile([P, 1], F32)
            nc.vector.tensor_copy(out=bcol0[:], in_=bcol0_i[:])

            s0_ps = pps.tile([P, 1], F32, space="PSUM", tag="small_ps")
            nc.tensor.matmul(out=s0_ps[:], lhsT=tri[:], rhs=bcol0[:],
                             start=True, stop=True)
            # t<128 => idx < 128 <= NCH-1, no clamp needed on the fast path
            idx0_i = sp.tile([P, 1], I32)
            nc.vector.tensor_copy(out=idx0_i[:], in_=s0_ps[:])

            up0 = mp.tile([P, D], F32, tag="up")
            nc.gpsimd.indirect_dma_start(
                out=up0[:], out_offset=None, in_=z[:],
                in_offset=IndirectOffsetOnAxis(ap=idx0_i[:, 0:1], axis=0),
            )

            # ---- load b (natural layout) and p -----------------------------
            # NOTE: a direct W-layout load of b would need 4096 4-byte
            # descriptors; HWDGE generates ~1 desc / 5ns, so that load
            # monopolizes the sync ring's descriptor generator for ~22us and
            # delays p and every store behind it. Load natural (32 x 512B
            # descriptors) and PE-transpose instead.
            b2d = b[:].rearrange("(j c) -> j c", c=P)          # [32, 128] DRAM view
            b_nat_i = sp.tile([NCOL, P], I32)
            nc.sync.dma_start(out=b_nat_i[:], in_=b2d)
            p2d = p[:].rearrange("(j c) -> j c", c=P)
            p_nat = sp.tile([NCOL, P], F32)
            nc.sync.dma_start(out=p_nat[:], in_=p2d)
            # bprev2[r, u] = b[128*(NT-NTAIL+u) + r - 1] for the tail tiles'
            # rolled-gather indices (idx[t-1] = s[t] - b[t-1])
            bprev2_i = sp.tile([P, NTAIL], I32)
            nc.sync.dma_start(
                out=bprev2_i[:],
                in_=b[(NT - NTAIL) * P - 1 : NT * P - 1].rearrange(
                    "(u c) -> c u", c=P
                ),
            )

            b_nat = sp.tile([NCOL, P], F32)
            nc.vector.tensor_copy(out=b_nat[:], in_=b_nat_i[:])
            bprev2 = sp.tile([P, NTAIL], F32)
            nc.vector.tensor_copy(out=bprev2[:], in_=bprev2_i[:])

            # PE transpose to W layout [128, 32]: (c, j) = t = 128j + c
            bw_ps = pps.tile([P, NCOL], F32, space="PSUM", tag="small_ps")
            nc.tensor.transpose(out=bw_ps[:], in_=b_nat[:], identity=identf[:])
            b_w = sp.tile([P, NCOL], F32)
            nc.vector.tensor_copy(out=b_w[:], in_=bw_ps[:])

            allones = sp.tile([P, P], F32)
            nc.vector.memset(allones[:], 1.0)
            ones_row_f = sp.tile([1, P], F32)
            nc.vector.memset(ones_row_f[:], 1.0)

            # ---- s = partition-scan + column-offset broadcast (one PSUM) ---
            # M[i, j] = sum_{j' < j} b[128j' + i]  (contraction over the 32
            # natural-layout partitions), then colofs[j] = sum_i M[i, j],
            # broadcast to all partitions by an all-ones lhsT.
            m_ps = pps.tile([P, NCOL], F32, space="PSUM", tag="small_ps")
            nc.tensor.matmul(out=m_ps[:], lhsT=b_nat[:], rhs=tri32[:],
                             start=True, stop=True)
            m_sb = sp.tile([P, NCOL], F32)
            nc.vector.tensor_copy(out=m_sb[:], in_=m_ps[:])

            s_ps = pps.tile([P, NCOL], F32, space="PSUM", tag="small_ps")
            nc.tensor.matmul(out=s_ps[:], lhsT=tri[:], rhs=b_w[:],
                             start=True, stop=False)
            nc.tensor.matmul(out=s_ps[:], lhsT=allones[:], rhs=m_sb[:],
                             start=False, stop=True)

            # ---- gather indices: idx = min(s, NCH-1) -----------------------
            idx_i = sp.tile([P, NCOL], I32)
            nc.vector.tensor_scalar_min(out=idx_i[:], in0=s_ps[:],
                                        scalar1=float(NCH - 1))
            idx_f = sp.tile([P, NCOL], F32)
            nc.vector.tensor_scalar_min(out=idx_f[:], in0=s_ps[:],
                                        scalar1=float(NCH - 1))

            # tail tiles: idx_prev = min(s - b_prev, NCH-1) (s[t]-b[t-1]=s[t-1])
            sprev = sp.tile([P, NTAIL], F32)
            nc.vector.tensor_sub(out=sprev[:], in0=s_ps[:, NT - NTAIL : NT],
                                 in1=bprev2[:])
            idxp_i = sp.tile([P, NTAIL], I32)
            nc.vector.tensor_scalar_min(out=idxp_i[:], in0=sprev[:],
                                        scalar1=float(NCH - 1))

            # ---- p / q in W layout (off the gather critical path) ----------
            p_nat_l = sp.tile([NCOL, P], F32)
            nc.vector.tensor_copy(out=p_nat_l[:], in_=p_nat[:])
            pwt_ps = pps.tile([P, NCOL], F32, space="PSUM", tag="small_ps")
            nc.tensor.transpose(out=pwt_ps[:], in_=p_nat_l[:], identity=identf[:])
            p_w = sp.tile([P, NCOL], F32)
            nc.vector.tensor_copy(out=p_w[:], in_=pwt_ps[:])
            # out[0] = up[0] exactly: force p[0] = 1 so the blend is 1*up + 0*rolled
            nc.vector.memset(p_w[0:1, 0:1], 1.0)
            q_w = sp.tile([P, NCOL], F32)  # q = 1 - p
            nc.scalar.activation(
                out=q_w[:], in_=p_w[:],
                func=mybir.ActivationFunctionType.Copy, bias=1.0, scale=-1.0,
            )

            # ---- epilogue vectors for rows t = 128j ------------------------
            # bprev_row[j] = idx[128j - 1] (0 for j=0, harmless: q[0]=0).
            # Row 127 of idx_f is not a legal compute-engine base, so extract
            # it with a tiny SBUF->SBUF DMA, then rotate rows into columns
            # with [1,32]-lhsT matmuls against a single 1.0.
            bprev_row = sp.tile([1, NCOL], F32)
            nc.vector.memset(bprev_row[:], 0.0)
            nc.sync.dma_start(
                out=bprev_row[0:1, 1:NCOL], in_=idx_f[P - 1 : P, 0 : NCOL - 1]
            )

            cols_ps = pps.tile([NCOL, 4], F32, space="PSUM", tag="small_ps")
            for ci, row in enumerate([bprev_row, idx_f, p_w, q_w]):
                nc.tensor.matmul(
                    out=cols_ps[:, ci : ci + 1],
                    lhsT=row[0:1, 0:NCOL],
                    rhs=ones_row_f[0:1, 0:1],
                    start=True, stop=True,
                )
            bidx_i = sp.tile([NCOL, 1], I32)
            nc.vector.tensor_copy(out=bidx_i[:], in_=cols_ps[:, 0:1])
            fidx_i = sp.tile([NCOL, 1], I32)
            nc.vector.tensor_copy(out=fidx_i[:], in_=cols_ps[:, 1:2])
            pb_col = sp.tile([NCOL, 1], F32)
            nc.vector.tensor_copy(out=pb_col[:], in_=cols_ps[:, 2:3])
            qb_col = sp.tile([NCOL, 1], F32)
            nc.vector.tensor_copy(out=qb_col[:], in_=cols_ps[:, 3:4])

            # PE warm-up: the HAM clock gate keeps the PE at ~half clock
            # until it has been busy for ~4us. Burn that in at the tail of
            # setup, while the PE would otherwise idle waiting for the first
            # gather, so the main-loop matmuls run at full clock.
            warm_src = sp.tile([P, DH], F32)
            nc.vector.memset(warm_src[:], 1.0)
            for w in range(WARMUP_MM):
                wps = ppr.tile([P, DH], F32, space="PSUM", tag="roll")
                nc.tensor.matmul(out=wps[:], lhsT=ishift[:], rhs=warm_src[:],
                                 start=True, stop=True, skip_group_check=True)
                if w == WARMUP_MM - 1:
                    warm_sink = sp.tile([1, 1], F32)
                    nc.vector.tensor_copy(out=warm_sink[:], in_=wps[0:1, 0:1])

            # ---- main loop: gather, roll, blend, store ---------------------
            rolled_tail = {}
            for k in range(NT):
                if k == 0:
                    up = up0
                else:
                    up = mp.tile([P, D], F32, tag="up")
                    nc.gpsimd.indirect_dma_start(
                        out=up[:], out_offset=None, in_=z[:],
                        in_offset=IndirectOffsetOnAxis(ap=idx_i[:, k : k + 1], axis=0),
                    )

                # t1 = p * up on ACT (exact f32)
                t1 = mp.tile([P, D], F32, tag="t1")
                nc.scalar.mul(out=t1[:], in_=up[:], mul=p_w[:, k : k + 1])

                o = mp.tile([P, D], F32, tag="o")
                if k >= NT - NTAIL:
                    # tail tiles: rolled was HBM-gathered mid-stream so the
                    # final stores don't queue behind the PE matmul backlog
                    nc.vector.scalar_tensor_tensor(
                        out=o[:], in0=rolled_tail[k][:], scalar=q_w[:, k : k + 1],
                        in1=t1[:],
                        op0=mybir.AluOpType.mult, op1=mybir.AluOpType.add,
                    )
                else:
                    # rolled[i] = up[i-1] via PE shifted-identity (f32, exact;
                    # row 0 -> 0, fixed by the epilogue)
                    rps = ppr.tile([P, D], F32, space="PSUM", tag="roll")
                    for h in range(2):
                        sl = slice(h * DH, (h + 1) * DH)
                        nc.tensor.matmul(out=rps[:, sl], lhsT=ishift[:],
                                         rhs=up[:, sl],
                                         start=True, stop=True,
                                         skip_group_check=True)
                    nc.vector.scalar_tensor_tensor(
                        out=o[:], in0=rps[:], scalar=q_w[:, k : k + 1],
                        in1=t1[:],
                        op0=mybir.AluOpType.mult, op1=mybir.AluOpType.add,
                    )

                nc.sync.dma_start(out=out[k * P : (k + 1) * P, :], in_=o[:])

                if k == 8:
                    # epilogue gathers + blend, issued mid-loop so they fill
                    # gather-stream slack instead of delaying tile 0 (gpsimd
                    # FIFO) or extending the tail; only the store is last.
                    upf = sp.tile([NCOL, D], F32)
                    nc.gpsimd.indirect_dma_start(
                        out=upf[:], out_offset=None, in_=z[:],
                        in_offset=IndirectOffsetOnAxis(ap=fidx_i[:, 0:1], axis=0),
                    )
                    rollf = sp.tile([NCOL, D], F32)
                    nc.gpsimd.indirect_dma_start(
                        out=rollf[:], out_offset=None, in_=z[:],
                        in_offset=IndirectOffsetOnAxis(ap=bidx_i[:, 0:1], axis=0),
                    )
                    t1b = sp.tile([NCOL, D], F32)
                    nc.scalar.mul(out=t1b[:], in_=upf[:], mul=pb_col[:])
                    ob = sp.tile([NCOL, D], F32)
                    nc.vector.scalar_tensor_tensor(
                        out=ob[:], in0=rollf[:], scalar=qb_col[:], in1=t1b[:],
                        op0=mybir.AluOpType.mult, op1=mybir.AluOpType.add,
                    )
                if k == 10:
                    # tail rolled-gathers, issued mid-stream: the data sits in
                    # SBUF until tiles NT-NTAIL.. blend against it.
                    for u in range(NTAIL):
                        rt = sp.tile([P, D], F32)
                        nc.gpsimd.indirect_dma_start(
                            out=rt[:], out_offset=None, in_=z[:],
                            in_offset=IndirectOffsetOnAxis(
                                ap=idxp_i[:, u : u + 1], axis=0
                            ),
                        )
                        rolled_tail[NT - NTAIL + u] = rt

            # ---- epilogue store: redo rows t = 128j exactly ----------------
            # Same HWDGE queue as the main stores, so FIFO order makes this
            # overwrite win.
            out_rows0 = out[:].rearrange("(j r) d -> j r d", r=P)[:, 0:1, :]
            nc.sync.dma_start(out=out_rows0, in_=ob[:, None, :])

    # Run the Bacc lowering passes (register allocation, event-semaphore
    # splitting, ...) -- run_bass_via_pjrt serializes nc.m as-is.
    nc.finalize()
    return nc


_NC_CACHE = None


def _get_nc() -> bass.Bass:
    global _NC_CACHE
    if _NC_CACHE is None:
        _NC_CACHE = build_bass()
    return _NC_CACHE


def make_in_maps(z: np.ndarray, p: np.ndarray, b: np.ndarray) -> list[dict]:
    return [
        {
            "z": np.ascontiguousarray(z[i], dtype=np.float32),
            "p": np.ascontiguousarray(p[i], dtype=np.float32),
            "b": np.ascontiguousarray(b[i], dtype=np.int32),
        }
        for i in range(B)
    ]


def kernel(z, p, b, original_len=None, **_unused) -> np.ndarray:
    z = np.asarray(z, dtype=np.float32)
    p = np.asarray(p, dtype=np.float32)
    b = np.asarray(b, dtype=np.int32)
    assert z.shape == (B, NCH, D) and p.shape == (B, T) and b.shape == (B, T)

    nc = _get_nc()
    res = run_bass_kernel_spmd(nc, make_in_maps(z, p, b), list(range(B)))
    return np.stack([r["out"] for r in res.results], axis=0)
